# revision 15
# baseline (speedup 1.0000x reference)
"""Trainium2 Bass kernel for MultiHeadAttention with shape-bias penalty.

Strategy: data-parallel over batch (16 batches -> 2 per core on 8 cores).
Each core runs the full per-batch computation:
  qh/kh/vh projections -> logits (transposed: S^T[j,i]) -> subtract symmetric
  penalty -> exp (no max-subtraction; logits are bounded) -> O accumulation
  with a ones-column appended to V so row sums fall out of the matmul ->
  per-partition normalize -> out projection + residual -> LayerNorm.

Self-contained: builds the Bass module lazily, shards inputs, runs via
run_bass_kernel_spmd, reassembles the full output.
"""

import json

import numpy as np

import concourse.bass as bass
import concourse.mybir as mybir
import concourse.tile as tile
from concourse.bass_utils import run_bass_kernel_spmd
from concourse.masks import make_identity
from concourse.vector_clock import ScopedClock

# ---------------------------------------------------------------------------
# Problem constants (hardcoded per spec)
# ---------------------------------------------------------------------------
B = 16
N_CORES = 8
B_LOC = B // N_CORES  # batches per core
N = 577  # sequence length (1 CLS + 576 patches)
D = 768  # model dim
H = 12  # heads
DK = 64  # head dim
TEMP = float(DK) ** 0.5
LN_EPS = 1e-6
EPS = 1e-12
P = 128  # partitions

NCH = (N + P - 1) // P  # 5 token chunks: 128,128,128,128,65
CH_SZ = [min(P, N - c * P) for c in range(NCH)]
NDC = D // P  # 6 d-chunks
IFR = [(0, 512), (512, 65)]  # free-dim split of N for matmul N<=512
FP = mybir.dt.float32

MAX_WAITS_PER_INST = 1


# ---------------------------------------------------------------------------
# Walrus compatibility: this toolchain accepts at most one sem wait per
# instruction; Tile can attach several. Split extras onto injected NoOps
# (same engine, in-order) in the serialized BIR, and chain the final drain.
# ---------------------------------------------------------------------------
def _split_multiwait_bir(bir_bytes: bytes) -> bytes:
    bir = json.loads(bir_bytes)
    counter = [0]
    for f in bir["functions"]:
        for blk in f["blocks"]:
            new = []
            for inst in blk["instructions"]:
                si = inst.get("sync_info")
                if si:
                    waits = si.get("on_wait") or []
                    if len(waits) > MAX_WAITS_PER_INST:
                        for w in waits[:-MAX_WAITS_PER_INST]:
                            counter[0] += 1
                            new.append(
                                {
                                    "debug": inst.get("debug", 0),
                                    "engine": inst["engine"],
                                    "ins": [],
                                    "name": f"I-wsplit-{counter[0]}",
                                    "opcode": "NoOp",
                                    "outs": [],
                                    "sync_info": {"on_update": [], "on_wait": [w]},
                                }
                            )
                        si["on_wait"] = waits[-MAX_WAITS_PER_INST:]
                    ups = si.get("on_update") or []
                    if len(ups) > 1 and inst["opcode"] not in (
                        "DMACopy",
                        "DMATranspose",
                        "TriggeredCopy",
                    ):
                        for u in ups[1:]:
                            counter[0] += 1
                            new.append(inst)
                            inst = {
                                "debug": inst.get("debug", 0),
                                "engine": inst["engine"],
                                "ins": [],
                                "name": f"I-usplit-{counter[0]}",
                                "opcode": "NoOp",
                                "outs": [],
                                "sync_info": {"on_update": [u], "on_wait": []},
                            }
                        si["on_update"] = [ups[0]]
                new.append(inst)
            blk["instructions"] = new
    return json.dumps(bir).encode()


class _SplitDrainTileContext(tile.TileContext):
    def _drain_and_barrier(self, tick_clock, wait_clock):
        import bass_rust as _bass_rust

        drain_inst = self.nc.sync.drain()
        wait_clock.add_sem_waits(
            drain_inst.ins, ScopedClock({None: tick_clock.global_clock})
        )
        waits = list(drain_inst.ins.sync_info.on_wait)
        if len(waits) > MAX_WAITS_PER_INST:
            si = drain_inst.ins.sync_info
            si.on_wait = waits[:MAX_WAITS_PER_INST]
            drain_inst.ins.sync_info = si
            for i in range(MAX_WAITS_PER_INST, len(waits), MAX_WAITS_PER_INST):
                extra = self.nc.sync.drain()
                extra.ins.sync_info = _bass_rust.SyncInfo(
                    on_wait=waits[i : i + MAX_WAITS_PER_INST], on_update=[]
                )
        self.nc.all_engine_barrier()
        assert self.sems is not None
        popped = self.nc._tile_sem_poison_stack.pop()
        assert popped is self._sem_poison
        self.nc.clear_and_free_semaphores(list(self.sems.allocated().values()))
        self.nc.all_engine_barrier()


def _install_bir_postpass(nc):
    orig = nc.to_json_bytes
    nc.to_json_bytes = lambda: _split_multiwait_bir(orig())
    return nc


# ---------------------------------------------------------------------------
# Kernel body
# ---------------------------------------------------------------------------
def _ts(c):
    return slice(c * P, c * P + CH_SZ[c])


def build_kernel(n_batches=B_LOC):
    AF = mybir.ActivationFunctionType
    ALU = mybir.AluOpType

    nc = bass.Bass("TRN2", target_bir_lowering=False)
    dq = nc.dram_tensor("q", [n_batches, N, D], FP, kind="ExternalInput")
    dk = nc.dram_tensor("k", [n_batches, N, D], FP, kind="ExternalInput")
    dv = nc.dram_tensor("v", [n_batches, N, D], FP, kind="ExternalInput")
    dpos = nc.dram_tensor("pos", [n_batches, N - 1, 2], FP, kind="ExternalInput")
    demb = nc.dram_tensor("emb", [n_batches, N, D], FP, kind="ExternalInput")
    dwq = nc.dram_tensor("w_qs", [D, D], FP, kind="ExternalInput")
    dwk = nc.dram_tensor("w_ks", [D, D], FP, kind="ExternalInput")
    dwv = nc.dram_tensor("w_vs", [D, D], FP, kind="ExternalInput")
    dwf = nc.dram_tensor("w_fc", [D, D], FP, kind="ExternalInput")
    dgam = nc.dram_tensor("gamma", [D], FP, kind="ExternalInput")
    dbet = nc.dram_tensor("beta", [D], FP, kind="ExternalInput")
    dout = nc.dram_tensor("out", [n_batches, N, D], FP, kind="ExternalOutput")

    with _SplitDrainTileContext(nc) as tc:
        _kernel_body(
            tc, nc, AF, ALU, n_batches,
            dq, dk, dv, dpos, demb, dwq, dwk, dwv, dwf, dgam, dbet, dout,
        )
    _install_bir_postpass(nc)
    return nc


def _kernel_body(
    tc, nc, AF, ALU, n_batches,
    dq, dk, dv, dpos, demb, dwq, dwk, dwv, dwf, dgam, dbet, dout,
):
    from contextlib import ExitStack

    with ExitStack() as ctx:
        singles = ctx.enter_context(tc.tile_pool(name="singles", bufs=1))
        wpool = ctx.enter_context(tc.tile_pool(name="w", bufs=2))
        xin = ctx.enter_context(tc.tile_pool(name="xin", bufs=1))
        xc = ctx.enter_context(tc.tile_pool(name="xc", bufs=6))
        xT = ctx.enter_context(tc.tile_pool(name="xT", bufs=1))
        hT = ctx.enter_context(tc.tile_pool(name="hT", bufs=2))
        vpool = ctx.enter_context(tc.tile_pool(name="vp", bufs=1))
        penp = ctx.enter_context(tc.tile_pool(name="pen", bufs=1))
        epool = ctx.enter_context(tc.tile_pool(name="E", bufs=6))
        scr = ctx.enter_context(tc.tile_pool(name="scr", bufs=2))
        small = ctx.enter_context(tc.tile_pool(name="small", bufs=4))
        outp = ctx.enter_context(tc.tile_pool(name="outp", bufs=2))
        pt = ctx.enter_context(tc.tile_pool(name="pt", bufs=2, space="PSUM"))
        pa = ctx.enter_context(tc.tile_pool(name="pa", bufs=4, space="PSUM"))
        po = ctx.enter_context(tc.tile_pool(name="po", bufs=2, space="PSUM"))

        # ---- constants ----
        ident = singles.tile([P, P], FP)
        make_identity(nc, ident)
        ones_row = singles.tile([1, N], FP)
        nc.vector.memset(ones_row, 1.0)
        eps12 = singles.tile([P, 1], FP)
        nc.vector.memset(eps12, EPS)
        epsln = singles.tile([P, 1], FP)
        nc.vector.memset(epsln, LN_EPS)
        gam_b = singles.tile([P, D], FP)
        nc.sync.dma_start(out=gam_b, in_=dgam[None, :].to_broadcast((P, D)))
        bet_b = singles.tile([P, D], FP)
        nc.sync.dma_start(out=bet_b, in_=dbet[None, :].to_broadcast((P, D)))

        def transpose_chunks(src_chunks, ident_ap, tag):
            """src_chunks: list of ([sz, D] tile) -> dst [p, NDC, N] d-major."""
            dst = xT.tile([P, NDC, N], FP, tag="xT")
            for c in range(NCH):
                sz = CH_SZ[c]
                for dc in range(NDC):
                    ps = pt.tile([P, P], FP, tag="ps")
                    nc.tensor.transpose(
                        ps[0:P, 0:sz],
                        src_chunks[c][0:sz, dc * P : (dc + 1) * P],
                        ident_ap[0:sz, 0:sz],
                    )
                    nc.vector.tensor_copy(dst[:, dc, _ts(c)], ps[0:P, 0:sz])
            return dst

        def load_chunks(dram, b):
            chunks = []
            for c in range(NCH):
                sz = CH_SZ[c]
                t = xc.tile([P, D], FP, tag="xc")
                nc.sync.dma_start(out=t[0:sz, :], in_=dram[b, _ts(c), :])
                chunks.append(t)
            return chunks

        for b in range(n_batches):
            # ================= load + transpose inputs =================
            # qin lives until the residual add at the end of the batch
            qin = xin.tile([P, NCH, D], FP, tag="qin", bufs=1)
            nc.sync.dma_start(
                out=qin[:, 0 : NCH - 1, :],
                in_=dq[b, 0 : (NCH - 1) * P, :].rearrange("(c p) d -> p c d", p=P),
            )
            nc.sync.dma_start(
                out=qin[0 : CH_SZ[NCH - 1], NCH - 1, :],
                in_=dq[b, (NCH - 1) * P : N, :],
            )
            xqT = transpose_chunks([qin[:, c, :] for c in range(NCH)], ident, "xT")

            wq = wpool.tile([P, NDC, D], FP, tag="w")
            nc.sync.dma_start(out=wq, in_=dwq.ap().rearrange("(c p) o -> p c o", p=P))
            wk = wpool.tile([P, NDC, D], FP, tag="w")
            nc.sync.dma_start(out=wk, in_=dwk.ap().rearrange("(c p) o -> p c o", p=P))

            # qhT: [o, n] = (x @ W)^T via lhsT=W chunk, rhs=xT
            qhT = hT.tile([P, NDC, N], FP, tag="hT")
            for oc in range(NDC):
                for i0, isz in IFR:
                    psm = pa.tile([P, 512], FP, tag="pa")
                    for dc in range(NDC):
                        nc.tensor.matmul(
                            psm[0:P, 0:isz],
                            wq[:, dc, oc * P : (oc + 1) * P],
                            xqT[:, dc, i0 : i0 + isz],
                            start=(dc == 0),
                            stop=(dc == NDC - 1),
                        )
                    # fold 1/TEMP into the PSUM->SBUF move
                    nc.vector.tensor_scalar_mul(
                        qhT[:, oc, i0 : i0 + isz], psm[0:P, 0:isz], 1.0 / TEMP
                    )

            kch = load_chunks(dk, b)
            xkT = transpose_chunks(kch, ident, "xT")
            khT = hT.tile([P, NDC, N], FP, tag="hT")
            for oc in range(NDC):
                for i0, isz in IFR:
                    psm = pa.tile([P, 512], FP, tag="pa")
                    for dc in range(NDC):
                        nc.tensor.matmul(
                            psm[0:P, 0:isz],
                            wk[:, dc, oc * P : (oc + 1) * P],
                            xkT[:, dc, i0 : i0 + isz],
                            start=(dc == 0),
                            stop=(dc == NDC - 1),
                        )
                    nc.vector.tensor_copy(khT[:, oc, i0 : i0 + isz], psm[0:P, 0:isz])

            # vh (+ ones col): [n, h, 65]
            vch = load_chunks(dv, b)
            xvT = transpose_chunks(vch, ident, "xT")
            wv = wpool.tile([P, NDC, D], FP, tag="w")
            nc.sync.dma_start(out=wv, in_=dwv.ap().rearrange("(c p) o -> p c o", p=P))
            vh = vpool.tile([P, NCH, H, DK + 1], FP, tag="vh")
            nc.vector.memset(vh[:, :, :, DK : DK + 1], 1.0)
            for c in range(NCH):
                sz = CH_SZ[c]
                for og in range(2):
                    psm = pa.tile([P, 512], FP, tag="pa")
                    for dc in range(NDC):
                        nc.tensor.matmul(
                            psm[0:sz, 0:384],
                            xvT[:, dc, _ts(c)],
                            wv[:, dc, og * 384 : (og + 1) * 384],
                            start=(dc == 0),
                            stop=(dc == NDC - 1),
                        )
                    nc.vector.tensor_copy(
                        vh[0:sz, c, og * 6 : (og + 1) * 6, 0:DK],
                        psm[0:sz, 0:384].rearrange("p (h e) -> p h e", h=6),
                    )

            # ---- patch embeddings: normalize rows, zero CLS, transpose ----
            ech = load_chunks(demb, b)
            dump = scr.tile([P, D], FP, tag="dump", bufs=1)
            for c in range(NCH):
                sz = CH_SZ[c]
                nsum = small.tile([P, 1], FP, tag="nsum")
                nc.scalar.activation(
                    out=dump[0:sz],
                    in_=ech[c][0:sz, :],
                    func=AF.Square,
                    accum_out=nsum[0:sz],
                )
                # sqrt(sum + EPS); EPS folded into the bias
                nc.scalar.activation(
                    out=nsum[0:sz], in_=nsum[0:sz], func=AF.Sqrt, bias=eps12[0:sz]
                )
                rin = small.tile([P, 1], FP, tag="rin")
                nc.vector.reciprocal(rin[0:sz], nsum[0:sz])
                nc.vector.tensor_scalar_mul(ech[c][0:sz, :], ech[c][0:sz, :], rin[0:sz])
            nc.vector.memset(ech[0][0:1, :], 0.0)  # zero CLS row -> zero pen row/col
            eT = transpose_chunks(ech, ident, "xT")

            # ---- positions (+CLS row zero) and |p|^2 ----
            ptile = xin.tile([P, NCH, 3], FP, tag="ptile", bufs=1)
            nc.vector.memset(ptile, 0.0)
            nc.sync.dma_start(out=ptile[1:P, 0, 0:2], in_=dpos[b, 0 : P - 1, :])
            nc.sync.dma_start(
                out=ptile[:, 1 : NCH - 1, 0:2],
                in_=dpos[b, P - 1 : (NCH - 1) * P - 1, :].rearrange(
                    "(c p) d -> p c d", p=P
                ),
            )
            nc.sync.dma_start(
                out=ptile[0 : CH_SZ[NCH - 1], NCH - 1, 0:2],
                in_=dpos[b, (NCH - 1) * P - 1 : N - 1, :],
            )
            for c in range(NCH):
                sz = CH_SZ[c]
                sq2 = small.tile([P, 2], FP, tag="sq2")
                nc.scalar.activation(
                    out=sq2[0:sz],
                    in_=ptile[0:sz, c, 0:2],
                    func=AF.Square,
                    accum_out=ptile[0:sz, c, 2:3],
                )
            posT = xin.tile([2, N], FP, tag="posT", bufs=1)
            p2T = xin.tile([1, N], FP, tag="p2T", bufs=1)
            for c in range(NCH):
                sz = CH_SZ[c]
                ps = pt.tile([P, P], FP, tag="ps")
                nc.tensor.transpose(
                    ps[0:2, 0:sz], ptile[0:sz, c, 0:2], ident[0:sz, 0:sz]
                )
                nc.vector.tensor_copy(posT[0:2, _ts(c)], ps[0:2, 0:sz])
                ps2 = pt.tile([P, P], FP, tag="ps")
                nc.tensor.transpose(
                    ps2[0:1, 0:sz], ptile[0:sz, c, 2:3], ident[0:sz, 0:sz]
                )
                nc.vector.tensor_copy(p2T[0:1, _ts(c)], ps2[0:1, 0:sz])
            pTm2 = xin.tile([2, N], FP, tag="pTm2", bufs=1)
            nc.scalar.mul(pTm2, posT[0:2, :], -2.0)

            # ================= penalty (symmetric, CLS row/col zero) ========
            pen = penp.tile([P, NCH, N], FP, tag="pen")
            for c in range(NCH):
                sz = CH_SZ[c]
                for j0, jsz in IFR:
                    # dist^2 = -2 p_i.p_j + |p_i|^2 + |p_j|^2  (3 matmuls)
                    psd = pa.tile([P, 512], FP, tag="pa")
                    nc.tensor.matmul(
                        psd[0:sz, 0:jsz],
                        posT[0:2, _ts(c)],
                        pTm2[0:2, j0 : j0 + jsz],
                        start=True,
                        stop=False,
                    )
                    nc.tensor.matmul(
                        psd[0:sz, 0:jsz],
                        p2T[0:1, _ts(c)],
                        ones_row[0:1, j0 : j0 + jsz],
                        start=False,
                        stop=False,
                    )
                    nc.tensor.matmul(
                        psd[0:sz, 0:jsz],
                        ones_row[0:1, _ts(c)],
                        p2T[0:1, j0 : j0 + jsz],
                        start=False,
                        stop=True,
                    )
                    dist = scr.tile([P, 512], FP, tag="dist")
                    nc.vector.tensor_scalar_max(
                        dist[0:sz, 0:jsz], psd[0:sz, 0:jsz], 0.0
                    )
                    nc.scalar.activation(
                        out=dist[0:sz, 0:jsz],
                        in_=dist[0:sz, 0:jsz],
                        func=AF.Sqrt,
                        bias=eps12[0:sz],
                    )
                    # sim = e . e^T (e pre-normalized, CLS row zeroed)
                    pss = pa.tile([P, 512], FP, tag="pa")
                    for dc in range(NDC):
                        nc.tensor.matmul(
                            pss[0:sz, 0:jsz],
                            eT[:, dc, _ts(c)],
                            eT[:, dc, j0 : j0 + jsz],
                            start=(dc == 0),
                            stop=(dc == NDC - 1),
                        )
                    nc.vector.tensor_tensor(
                        out=pen[0:sz, c, j0 : j0 + jsz],
                        in0=pss[0:sz, 0:jsz],
                        in1=dist[0:sz, 0:jsz],
                        op=ALU.mult,
                    )

            # ================= attention =================
            # oT is filled head-by-head with normalized, transposed output
            oT = xT.tile([P, NDC, N], FP, tag="xT")
            for h in range(H):
                oc, orow = h // 2, (h % 2) * DK
                e_tiles = []
                for c in range(NCH):
                    sz = CH_SZ[c]
                    et = epool.tile([P, N], FP, tag="E")
                    for i0, isz in IFR:
                        pss = pa.tile([P, 512], FP, tag="pa")
                        nc.tensor.matmul(
                            pss[0:sz, 0:isz],
                            khT[orow : orow + DK, oc, _ts(c)],
                            qhT[orow : orow + DK, oc, i0 : i0 + isz],
                            start=True,
                            stop=True,
                        )
                        nc.vector.tensor_tensor(
                            out=et[0:sz, i0 : i0 + isz],
                            in0=pss[0:sz, 0:isz],
                            in1=pen[0:sz, c, i0 : i0 + isz],
                            op=ALU.subtract,
                        )
                        nc.scalar.activation(
                            out=et[0:sz, i0 : i0 + isz],
                            in_=et[0:sz, i0 : i0 + isz],
                            func=AF.Exp,
                        )
                    e_tiles.append(et)
                for ic in range(NCH):
                    szi = CH_SZ[ic]
                    pso = po.tile([P, DK + 1], FP, tag="po")
                    for c in range(NCH):
                        sz = CH_SZ[c]
                        nc.tensor.matmul(
                            pso[0:szi, :],
                            e_tiles[c][0:sz, ic * P : ic * P + szi],
                            vh[0:sz, c, h, :],
                            start=(c == 0),
                            stop=(c == NCH - 1),
                        )
                    rcp = small.tile([P, 1], FP, tag="rcp")
                    nc.vector.reciprocal(rcp[0:szi], pso[0:szi, DK : DK + 1])
                    on = small.tile([P, DK], FP, tag="on")
                    nc.vector.tensor_scalar_mul(
                        on[0:szi], pso[0:szi, 0:DK], rcp[0:szi]
                    )
                    pst = pt.tile([P, P], FP, tag="ps")
                    nc.tensor.transpose(
                        pst[0:DK, 0:szi], on[0:szi, 0:DK], ident[0:szi, 0:szi]
                    )
                    nc.vector.tensor_copy(
                        oT[orow : orow + DK, oc, ic * P : ic * P + szi],
                        pst[0:DK, 0:szi],
                    )

            # ================= output proj + residual + LN =================
            wf = wpool.tile([P, NDC, D], FP, tag="w")
            nc.sync.dma_start(out=wf, in_=dwf.ap().rearrange("(c p) o -> p c o", p=P))

            for c in range(NCH):
                sz = CH_SZ[c]
                xo = outp.tile([P, D], FP, tag="xo")
                for og in range(2):
                    psf = pa.tile([P, 512], FP, tag="pa")
                    for dc in range(NDC):
                        nc.tensor.matmul(
                            psf[0:sz, 0:384],
                            oT[:, dc, _ts(c)],
                            wf[:, dc, og * 384 : (og + 1) * 384],
                            start=(dc == 0),
                            stop=(dc == NDC - 1),
                        )
                    nc.vector.tensor_add(
                        xo[0:sz, og * 384 : (og + 1) * 384],
                        psf[0:sz, 0:384],
                        qin[0:sz, c, og * 384 : (og + 1) * 384],
                    )
                # LayerNorm over D
                stats = small.tile([P, 2, 6], FP, tag="stats")
                for g in range(2):
                    nc.vector.bn_stats(
                        stats[0:sz, g, :], xo[0:sz, g * 384 : (g + 1) * 384]
                    )
                mv = small.tile([P, 2], FP, tag="mv")
                nc.vector.bn_aggr(mv[0:sz], stats[0:sz])
                rstd = small.tile([P, 1], FP, tag="rstd")
                nc.scalar.activation(
                    out=rstd[0:sz], in_=mv[0:sz, 1:2], func=AF.Sqrt, bias=epsln[0:sz]
                )
                nc.vector.reciprocal(rstd[0:sz], rstd[0:sz])
                yt = outp.tile([P, D], FP, tag="yt")
                nc.vector.tensor_scalar(
                    out=yt[0:sz],
                    in0=xo[0:sz],
                    scalar1=mv[0:sz, 0:1],
                    scalar2=rstd[0:sz],
                    op0=ALU.subtract,
                    op1=ALU.mult,
                )
                nc.vector.tensor_mul(yt[0:sz], yt[0:sz], gam_b[0:sz])
                nc.vector.tensor_add(yt[0:sz], yt[0:sz], bet_b[0:sz])
                nc.sync.dma_start(out=dout[b, _ts(c), :], in_=yt[0:sz])


# ---------------------------------------------------------------------------
# Host-side entry point
# ---------------------------------------------------------------------------
_NC_CACHE = {}


def _get_nc():
    if "nc" not in _NC_CACHE:
        _NC_CACHE["nc"] = build_kernel(B_LOC)
    return _NC_CACHE["nc"]


def _make_in_maps(q, k, v, patch_positions, patch_embeddings,
                  w_qs, w_ks, w_vs, w_fc, ln_gamma, ln_beta):
    f32 = lambda x: np.ascontiguousarray(np.asarray(x), dtype=np.float32)
    shared = {
        "w_qs": f32(w_qs), "w_ks": f32(w_ks), "w_vs": f32(w_vs),
        "w_fc": f32(w_fc), "gamma": f32(ln_gamma), "beta": f32(ln_beta),
    }
    in_maps = []
    for c in range(N_CORES):
        sl = slice(c * B_LOC, (c + 1) * B_LOC)
        in_maps.append(
            {
                "q": f32(q[sl]),
                "k": f32(k[sl]),
                "v": f32(v[sl]),
                "pos": f32(patch_positions[sl]),
                "emb": f32(patch_embeddings[sl]),
                **shared,
            }
        )
    return in_maps


def kernel(q, k, v, patch_positions, patch_embeddings,
           w_qs, w_ks, w_vs, w_fc, ln_gamma, ln_beta):
    nc = _get_nc()
    in_maps = _make_in_maps(q, k, v, patch_positions, patch_embeddings,
                            w_qs, w_ks, w_vs, w_fc, ln_gamma, ln_beta)
    res = run_bass_kernel_spmd(nc, in_maps, core_ids=list(range(N_CORES)))
    return np.concatenate([r["out"] for r in res.results], axis=0)
